# revision 15
# baseline (speedup 1.0000x reference)
"""Sparse-attention kernel for TRN2 (8 NeuronCores, data-parallel over batch).

Reference computation (per batch b):
    S = X @ X.T / sqrt(D)                 # [N, N]
    E = exp(S) * m[:, None] * m[None, :]  # bidirectional mask
    out = (E @ X) / (rowsum(E) + EPS)

Kernel strategy per core (one batch element per core, B == n_cores == 8):
  - X [N, D] f32 is DMA'd in, cast to bf16; X^T built with PE transposes.
  - S computed blockwise with bf16 matmuls accumulating in PSUM f32.
  - exp applied on ScalarE with scale=1/sqrt(D) and a per-partition bias that
    is 0 for live rows and -30000 for masked rows (exp underflows to +0.0),
    which applies the row mask for free.
  - A single fused DVE tensor_tensor_reduce multiplies by the broadcast
    column mask AND accumulates the row sums.  The resulting masked
    E' = m_i * m_j * exp(S_ij) is symmetric, so its [row-part, col-free]
    tiles serve directly as lhsT (= E'^T tiles) for the second matmul.
  - U = E' @ X accumulated in PSUM f32; final per-row scale by
    1/(rowsum + EPS) on DVE, then DMA out in f32.
"""

import numpy as np

import concourse.bass as bass
import concourse.bacc as bacc
import concourse.mybir as mybir
from concourse.tile import TileContext
from concourse.masks import make_identity

B = 8
N = 2048
D = 1024
P = 128
NT = N // P     # 16 row/col blocks of S
KT = D // P     # 8 contraction blocks for S = X X^T
FREE = 512      # matmul moving-operand free dim / PSUM bank width (f32)
NJ = N // FREE  # 4 column chunks of S per row block
DC = D // FREE  # 2 output column chunks
EPS = 1e-7
INV_SQRT_D = 1.0 / float(np.sqrt(D))

F32 = mybir.dt.float32
BF16 = mybir.dt.bfloat16


def build_nc(finalize=True):
    # Bacc (not raw Bass): its compile() pass legalizes multi-wait
    # instructions into event semaphores, which walrus requires.
    nc = bacc.Bacc()
    # x arrives pre-cast to bf16 from the host (input marshaling; the kernel
    # computes in bf16 anyway and DMA cannot cast).
    x_ext = nc.declare_dram_parameter("x", [N, D], BF16, isOutput=False)
    m_ext = nc.declare_dram_parameter("maskf", [N], F32, isOutput=False)
    out_ext = nc.declare_dram_parameter("out", [N, D], F32, isOutput=True)

    with TileContext(nc) as tc:
        with (
            tc.tile_pool(name="persist", bufs=1) as persist,
            tc.tile_pool(name="outp", bufs=4) as outp,
            tc.tile_pool(name="psum", bufs=8, space="PSUM") as psum,
        ):
            # ---------------- constants ----------------
            ident = persist.tile([P, P], BF16, name="ident")
            make_identity(nc, ident)

            # mrow[p, t] = mask[t*P + p]
            mrow = persist.tile([P, NT], F32, name="mrow")
            nc.sync.dma_start(out=mrow, in_=m_ext[:].rearrange("(t p) -> p t", p=P))

            # additive row-mask bias for exp: 0 where mask=1, -30000 where 0
            mbias = persist.tile([P, NT], F32, name="mbias")
            nc.vector.tensor_scalar(
                out=mbias, in0=mrow, scalar1=-1.0, scalar2=30000.0,
                op0=mybir.AluOpType.add, op1=mybir.AluOpType.mult,
            )

            # column mask broadcast across partitions, cast to bf16
            # column-mask broadcast across partitions via PE outer product
            # ones[128,1] @ m[1,N] (multi-partition broadcast DMAs trip a
            # walrus "Too many sync wait commands" limit, so avoid them).
            m1 = persist.tile([1, N], F32, name="m1")
            nc.sync.dma_start(out=m1, in_=m_ext[None, :])
            m1b = persist.tile([1, N], BF16, name="m1b")
            nc.vector.tensor_copy(out=m1b, in_=m1)
            ones1 = persist.tile([1, P], BF16, name="ones1")
            nc.vector.memset(ones1, 1.0)
            mbc = persist.tile([P, N], BF16, name="mbc")
            for c in range(NJ):
                pm = psum.tile([P, FREE], F32, name="ps_m", tag="ps")
                nc.tensor.matmul(
                    pm, lhsT=ones1, rhs=m1b[:, c * FREE:(c + 1) * FREE],
                    start=True, stop=True,
                )
                nc.vector.tensor_copy(out=mbc[:, c * FREE:(c + 1) * FREE], in_=pm)

            # ---------------- persistent tensors ----------------
            xbf = persist.tile([P, NT, D], BF16, name="xbf")   # X rows, bf16
            xt = persist.tile([P, KT, N], BF16, name="xt")     # X^T [d, n]
            et = persist.tile([P, NT, N], BF16, name="et")     # masked exp(S)
            s_parts = persist.tile([P, NT, NJ], F32, name="s_parts")
            rowscale = persist.tile([P, NT], F32, name="rowscale")

            # ---- phase A: load X (bf16), transpose into xt ----
            for t in range(NT):
                nc.sync.dma_start(out=xbf[:, t, :], in_=x_ext[t * P:(t + 1) * P, :])
                for k in range(KT):
                    pt = psum.tile([P, P], BF16, name="ps_t", tag="ps")
                    nc.tensor.transpose(pt, xbf[:, t, k * P:(k + 1) * P], ident)
                    nc.vector.tensor_copy(out=xt[:, k, t * P:(t + 1) * P], in_=pt)

            # ---- phase B: S blocks -> exp -> col-mask + rowsum ----
            for t in range(NT):
                ps_chunks = [
                    psum.tile([P, FREE], F32, name=f"ps_s{c}", tag="ps")
                    for c in range(NJ)
                ]
                for k in range(KT):
                    lhsT = xt[:, k, t * P:(t + 1) * P]
                    for c in range(NJ):
                        nc.tensor.matmul(
                            ps_chunks[c], lhsT=lhsT,
                            rhs=xt[:, k, c * FREE:(c + 1) * FREE],
                            start=(k == 0), stop=(k == KT - 1),
                        )
                for c in range(NJ):
                    esl = et[:, t, c * FREE:(c + 1) * FREE]
                    nc.scalar.activation(
                        out=esl, in_=ps_chunks[c],
                        func=mybir.ActivationFunctionType.Exp,
                        bias=mbias[:, t:t + 1], scale=INV_SQRT_D,
                    )
                    # (tensor_tensor_reduce is rejected by this runtime;
                    # InstTensorScalarPtr with is_scalar_tensor_tensor works)
                    nc.vector.scalar_tensor_tensor(
                        out=esl, in0=esl, scalar=1.0,
                        in1=mbc[:, c * FREE:(c + 1) * FREE],
                        op0=mybir.AluOpType.bypass,
                        op1=mybir.AluOpType.mult,
                        accum_out=s_parts[:, t, c:c + 1],
                    )
                nc.vector.reduce_sum(
                    out=rowscale[:, t:t + 1], in_=s_parts[:, t, :],
                    axis=mybir.AxisListType.X,
                )
                nc.vector.tensor_scalar_add(
                    out=rowscale[:, t:t + 1], in0=rowscale[:, t:t + 1], scalar1=EPS
                )
                nc.vector.reciprocal(
                    out=rowscale[:, t:t + 1], in_=rowscale[:, t:t + 1]
                )

            # ---- phase C: U = E' @ X, row-scale, store ----
            for b in range(NT):
                for c in range(DC):
                    pu = psum.tile([P, FREE], F32, name="ps_u", tag="ps")
                    for a in range(NT):
                        nc.tensor.matmul(
                            pu, lhsT=et[:, a, b * P:(b + 1) * P],
                            rhs=xbf[:, a, c * FREE:(c + 1) * FREE],
                            start=(a == 0), stop=(a == NT - 1),
                        )
                    ot = outp.tile([P, FREE], F32, name="ot", tag="ot")
                    nc.vector.tensor_scalar_mul(
                        out=ot, in0=pu, scalar1=rowscale[:, b:b + 1]
                    )
                    nc.gpsimd.dma_start(
                        out=out_ext[b * P:(b + 1) * P, c * FREE:(c + 1) * FREE],
                        in_=ot,
                    )
    if finalize:
        nc.finalize()
    return nc


_RUNNER = None


def _make_runner():
    """Compile the SPMD NEFF once; return f(x2d, m1d, zeros) -> out2d.

    Mirrors concourse.bass2jax.run_bass_via_pjrt's multi-core path (shard_map
    over 8 cores, per-core shard = BIR-declared shape), but keeps the jitted
    callable so repeat calls don't retrace/recompile, and skips output-buffer
    donation (this kernel writes every output element).
    """
    import jax
    from jax.sharding import Mesh, PartitionSpec
    from jax.experimental.shard_map import shard_map
    import concourse.mybir as mybir
    from concourse import bass2jax

    bass2jax.install_neuronx_cc_hook()
    nc = build_nc()
    assert nc.dbg_addr is None
    partition_name = nc.partition_id_tensor.name if nc.partition_id_tensor else None

    in_names, out_names, out_avals = [], [], []
    for alloc in nc.m.functions[0].allocations:
        if not isinstance(alloc, mybir.MemoryLocationSet):
            continue
        name = alloc.memorylocations[0].name
        if alloc.kind == "ExternalInput":
            if name != partition_name:
                in_names.append(name)
        elif alloc.kind == "ExternalOutput":
            out_names.append(name)
            out_avals.append(
                jax.core.ShapedArray(tuple(alloc.tensor_shape), mybir.dt.np(alloc.dtype))
            )
    n_params = len(in_names)
    all_names = in_names + out_names
    if partition_name is not None:
        all_names = all_names + [partition_name]

    def _body(*args):
        operands = list(args)
        if partition_name is not None:
            operands.append(bass2jax.partition_id_tensor())
        outs = bass2jax._bass_exec_p.bind(
            *operands,
            out_avals=tuple(out_avals),
            in_names=tuple(all_names),
            out_names=tuple(out_names),
            lowering_input_output_aliases=(),
            sim_require_finite=True,
            sim_require_nnan=True,
            nc=nc,
        )
        return tuple(outs)

    devices = jax.devices()[:B]
    mesh = Mesh(np.asarray(devices), ("core",))
    n_args = n_params + len(out_names)
    sharded = jax.jit(
        shard_map(
            _body,
            mesh=mesh,
            in_specs=(PartitionSpec("core"),) * n_args,
            out_specs=(PartitionSpec("core"),) * len(out_names),
            check_rep=False,
        ),
        keep_unused=True,
    )
    zeros = [np.zeros((B * a.shape[0], *a.shape[1:]), a.dtype) for a in out_avals]
    return sharded, zeros, [tuple(a.shape) for a in out_avals], in_names


def _get_runner():
    global _RUNNER
    if _RUNNER is None:
        _RUNNER = _make_runner()
    return _RUNNER


def _prep(x, mask):
    import ml_dtypes

    xb = np.ascontiguousarray(
        np.asarray(x, dtype=np.float32).astype(ml_dtypes.bfloat16)
    )
    maskf = np.ascontiguousarray(np.asarray(mask).astype(np.float32))
    assert xb.shape == (B, N, D) and maskf.shape == (B, N)
    # per-core shard of axis 0: concat over cores = just the 2D views
    return {"x": xb.reshape(B * N, D), "maskf": maskf.reshape(B * N)}


def kernel(x, mask):
    sharded, zeros, out_shapes, in_names = _get_runner()
    ins = _prep(x, mask)
    out_arrs = sharded(*[ins[n] for n in in_names], *zeros)
    out = np.asarray(out_arrs[0]).reshape(B, *out_shapes[0])
    return out


# revision 17
# speedup vs baseline: 614.5073x; 614.5073x over previous
"""Sparse-attention kernel for TRN2 (8 NeuronCores, data-parallel over batch).

Reference computation (per batch b):
    S = X @ X.T / sqrt(D)                 # [N, N]
    E = exp(S) * m[:, None] * m[None, :]  # bidirectional mask
    out = (E @ X) / (rowsum(E) + EPS)

Kernel strategy per core (one batch element per core, B == n_cores == 8):
  - X [N, D] f32 is DMA'd in, cast to bf16; X^T built with PE transposes.
  - S computed blockwise with bf16 matmuls accumulating in PSUM f32.
  - exp applied on ScalarE with scale=1/sqrt(D) and a per-partition bias that
    is 0 for live rows and -30000 for masked rows (exp underflows to +0.0),
    which applies the row mask for free.
  - A single fused DVE tensor_tensor_reduce multiplies by the broadcast
    column mask AND accumulates the row sums.  The resulting masked
    E' = m_i * m_j * exp(S_ij) is symmetric, so its [row-part, col-free]
    tiles serve directly as lhsT (= E'^T tiles) for the second matmul.
  - U = E' @ X accumulated in PSUM f32; final per-row scale by
    1/(rowsum + EPS) on DVE, then DMA out in f32.
"""

import numpy as np

import concourse.bass as bass
import concourse.bacc as bacc
import concourse.mybir as mybir
from concourse.tile import TileContext
from concourse.masks import make_identity

B = 8
N = 2048
D = 1024
P = 128
NT = N // P     # 16 row/col blocks of S
KT = D // P     # 8 contraction blocks for S = X X^T
FREE = 512      # matmul moving-operand free dim / PSUM bank width (f32)
NJ = N // FREE  # 4 column chunks of S per row block
DC = D // FREE  # 2 output column chunks
EPS = 1e-7
INV_SQRT_D = 1.0 / float(np.sqrt(D))

F32 = mybir.dt.float32
BF16 = mybir.dt.bfloat16


def build_nc(finalize=True):
    # Bacc (not raw Bass): its compile() pass legalizes multi-wait
    # instructions into event semaphores, which walrus requires.
    nc = bacc.Bacc()
    # x arrives pre-cast to bf16 from the host (input marshaling; the kernel
    # computes in bf16 anyway and DMA cannot cast).
    x_ext = nc.declare_dram_parameter("x", [N, D], BF16, isOutput=False)
    m_ext = nc.declare_dram_parameter("maskf", [N], F32, isOutput=False)
    out_ext = nc.declare_dram_parameter("out", [N, D], F32, isOutput=True)

    with TileContext(nc) as tc:
        with (
            tc.tile_pool(name="persist", bufs=1) as persist,
            tc.tile_pool(name="outp", bufs=4) as outp,
            tc.tile_pool(name="psum", bufs=8, space="PSUM") as psum,
        ):
            # ---------------- constants ----------------
            ident = persist.tile([P, P], BF16, name="ident")
            make_identity(nc, ident)

            # mrow[p, t] = mask[t*P + p]
            mrow = persist.tile([P, NT], F32, name="mrow")
            nc.sync.dma_start(out=mrow, in_=m_ext[:].rearrange("(t p) -> p t", p=P))

            # additive row-mask bias for exp: 0 where mask=1, -30000 where 0
            mbias = persist.tile([P, NT], F32, name="mbias")
            nc.vector.tensor_scalar(
                out=mbias, in0=mrow, scalar1=-1.0, scalar2=30000.0,
                op0=mybir.AluOpType.add, op1=mybir.AluOpType.mult,
            )

            # column mask broadcast across partitions, cast to bf16
            # column-mask broadcast across partitions via PE outer product
            # ones[128,1] @ m[1,N] (multi-partition broadcast DMAs trip a
            # walrus "Too many sync wait commands" limit, so avoid them).
            m1 = persist.tile([1, N], F32, name="m1")
            nc.sync.dma_start(out=m1, in_=m_ext[None, :])
            m1b = persist.tile([1, N], BF16, name="m1b")
            nc.vector.tensor_copy(out=m1b, in_=m1)
            ones1 = persist.tile([1, P], BF16, name="ones1")
            nc.vector.memset(ones1, 1.0)
            mbc = persist.tile([P, N], BF16, name="mbc")
            for c in range(NJ):
                pm = psum.tile([P, FREE], F32, name="ps_m", tag="ps")
                nc.tensor.matmul(
                    pm, lhsT=ones1, rhs=m1b[:, c * FREE:(c + 1) * FREE],
                    start=True, stop=True,
                )
                nc.vector.tensor_copy(out=mbc[:, c * FREE:(c + 1) * FREE], in_=pm)

            # ---------------- persistent tensors ----------------
            xbf = persist.tile([P, NT, D], BF16, name="xbf")   # X rows, bf16
            xt = persist.tile([P, KT, N], BF16, name="xt")     # X^T [d, n]
            et = persist.tile([P, NT, N], BF16, name="et")     # masked exp(S)
            s_parts = persist.tile([P, NT, NJ], F32, name="s_parts")
            rowscale = persist.tile([P, NT], F32, name="rowscale")

            # ---- phase A: load X (bf16), transpose into xt ----
            for t in range(NT):
                nc.sync.dma_start(out=xbf[:, t, :], in_=x_ext[t * P:(t + 1) * P, :])
                for k in range(KT):
                    pt = psum.tile([P, P], BF16, name="ps_t", tag="ps")
                    nc.tensor.transpose(pt, xbf[:, t, k * P:(k + 1) * P], ident)
                    nc.vector.tensor_copy(out=xt[:, k, t * P:(t + 1) * P], in_=pt)

            # ---- phase B: S blocks -> exp -> col-mask + rowsum ----
            for t in range(NT):
                ps_chunks = [
                    psum.tile([P, FREE], F32, name=f"ps_s{c}", tag="ps")
                    for c in range(NJ)
                ]
                for k in range(KT):
                    lhsT = xt[:, k, t * P:(t + 1) * P]
                    for c in range(NJ):
                        nc.tensor.matmul(
                            ps_chunks[c], lhsT=lhsT,
                            rhs=xt[:, k, c * FREE:(c + 1) * FREE],
                            start=(k == 0), stop=(k == KT - 1),
                        )
                for c in range(NJ):
                    esl = et[:, t, c * FREE:(c + 1) * FREE]
                    nc.scalar.activation(
                        out=esl, in_=ps_chunks[c],
                        func=mybir.ActivationFunctionType.Exp,
                        bias=mbias[:, t:t + 1], scale=INV_SQRT_D,
                    )
                    # (tensor_tensor_reduce is rejected by this runtime;
                    # InstTensorScalarPtr with is_scalar_tensor_tensor works)
                    nc.vector.scalar_tensor_tensor(
                        out=esl, in0=esl, scalar=1.0,
                        in1=mbc[:, c * FREE:(c + 1) * FREE],
                        op0=mybir.AluOpType.bypass,
                        op1=mybir.AluOpType.mult,
                        accum_out=s_parts[:, t, c:c + 1],
                    )
                nc.vector.reduce_sum(
                    out=rowscale[:, t:t + 1], in_=s_parts[:, t, :],
                    axis=mybir.AxisListType.X,
                )
                nc.vector.tensor_scalar_add(
                    out=rowscale[:, t:t + 1], in0=rowscale[:, t:t + 1], scalar1=EPS
                )
                nc.vector.reciprocal(
                    out=rowscale[:, t:t + 1], in_=rowscale[:, t:t + 1]
                )

            # ---- phase C: U = E' @ X, row-scale, store ----
            for b in range(NT):
                for c in range(DC):
                    pu = psum.tile([P, FREE], F32, name="ps_u", tag="ps")
                    for a in range(NT):
                        nc.tensor.matmul(
                            pu, lhsT=et[:, a, b * P:(b + 1) * P],
                            rhs=xbf[:, a, c * FREE:(c + 1) * FREE],
                            start=(a == 0), stop=(a == NT - 1),
                        )
                    ot = outp.tile([P, FREE], F32, name="ot", tag="ot")
                    nc.vector.tensor_scalar_mul(
                        out=ot, in0=pu, scalar1=rowscale[:, b:b + 1]
                    )
                    nc.gpsimd.dma_start(
                        out=out_ext[b * P:(b + 1) * P, c * FREE:(c + 1) * FREE],
                        in_=ot,
                    )
    if finalize:
        nc.finalize()
    return nc


_RUNNER = None


def _make_runner():
    """Compile the SPMD NEFF once; return f(x2d, m1d, zeros) -> out2d.

    Mirrors concourse.bass2jax.run_bass_via_pjrt's multi-core path (shard_map
    over 8 cores, per-core shard = BIR-declared shape), but keeps the jitted
    callable so repeat calls don't retrace/recompile, and skips output-buffer
    donation (this kernel writes every output element).
    """
    import jax
    from jax.sharding import Mesh, PartitionSpec
    from jax.experimental.shard_map import shard_map
    import concourse.mybir as mybir
    from concourse import bass2jax

    bass2jax.install_neuronx_cc_hook()
    nc = build_nc()
    assert nc.dbg_addr is None
    partition_name = nc.partition_id_tensor.name if nc.partition_id_tensor else None

    in_names, out_names, out_avals = [], [], []
    for alloc in nc.m.functions[0].allocations:
        if not isinstance(alloc, mybir.MemoryLocationSet):
            continue
        name = alloc.memorylocations[0].name
        if alloc.kind == "ExternalInput":
            if name != partition_name:
                in_names.append(name)
        elif alloc.kind == "ExternalOutput":
            out_names.append(name)
            out_avals.append(
                jax.core.ShapedArray(tuple(alloc.tensor_shape), mybir.dt.np(alloc.dtype))
            )
    n_params = len(in_names)
    all_names = in_names + out_names
    if partition_name is not None:
        all_names = all_names + [partition_name]

    def _body(*args):
        operands = list(args)
        if partition_name is not None:
            operands.append(bass2jax.partition_id_tensor())
        outs = bass2jax._bass_exec_p.bind(
            *operands,
            out_avals=tuple(out_avals),
            in_names=tuple(all_names),
            out_names=tuple(out_names),
            lowering_input_output_aliases=(),
            sim_require_finite=True,
            sim_require_nnan=True,
            nc=nc,
        )
        return tuple(outs)

    devices = jax.devices()[:B]
    mesh = Mesh(np.asarray(devices), ("core",))
    n_args = n_params + len(out_names)
    sharded = jax.jit(
        shard_map(
            _body,
            mesh=mesh,
            in_specs=(PartitionSpec("core"),) * n_args,
            out_specs=(PartitionSpec("core"),) * len(out_names),
            check_rep=False,
        ),
        keep_unused=True,
    )
    zeros = [np.zeros((B * a.shape[0], *a.shape[1:]), a.dtype) for a in out_avals]
    return sharded, zeros, [tuple(a.shape) for a in out_avals], in_names, mesh


def _get_runner():
    global _RUNNER
    if _RUNNER is None:
        _RUNNER = _make_runner()
    return _RUNNER


def _prep(x, mask):
    import ml_dtypes

    xb = np.ascontiguousarray(
        np.asarray(x, dtype=np.float32).astype(ml_dtypes.bfloat16)
    )
    maskf = np.ascontiguousarray(np.asarray(mask).astype(np.float32))
    assert xb.shape == (B, N, D) and maskf.shape == (B, N)
    # per-core shard of axis 0: concat over cores = just the 2D views
    return {"x": xb.reshape(B * N, D), "maskf": maskf.reshape(B * N)}


def kernel(x, mask):
    sharded, zeros, out_shapes, in_names, _mesh = _get_runner()
    ins = _prep(x, mask)
    out_arrs = sharded(*[ins[n] for n in in_names], *zeros)
    out = np.asarray(out_arrs[0]).reshape(B, *out_shapes[0])
    return out


# revision 25
# speedup vs baseline: 45399.2694x; 73.8791x over previous
"""Sparse-attention kernel for TRN2 (8 NeuronCores, data-parallel over batch).

Reference computation (per batch b):
    S = X @ X.T / sqrt(D)                 # [N, N]
    E = exp(S) * m[:, None] * m[None, :]  # bidirectional mask
    out = (E @ X) / (rowsum(E) + EPS)

Kernel strategy per core (one batch element per core, B == n_cores == 8):
  - X [N, D] f32 is DMA'd in, cast to bf16; X^T built with PE transposes.
  - S computed blockwise with bf16 matmuls accumulating in PSUM f32.
  - exp applied on ScalarE with scale=1/sqrt(D) and a per-partition bias that
    is 0 for live rows and -30000 for masked rows (exp underflows to +0.0),
    which applies the row mask for free.
  - A single fused DVE tensor_tensor_reduce multiplies by the broadcast
    column mask AND accumulates the row sums.  The resulting masked
    E' = m_i * m_j * exp(S_ij) is symmetric, so its [row-part, col-free]
    tiles serve directly as lhsT (= E'^T tiles) for the second matmul.
  - U = E' @ X accumulated in PSUM f32; final per-row scale by
    1/(rowsum + EPS) on DVE, then DMA out in f32.
"""

import numpy as np

import concourse.bass as bass
import concourse.bacc as bacc
import concourse.mybir as mybir
from concourse.tile import TileContext
from concourse.masks import make_identity

B = 8
N = 2048
D = 1024
P = 128
NT = N // P     # 16 row/col blocks of S
KT = D // P     # 8 contraction blocks for S = X X^T
FREE = 512      # matmul moving-operand free dim / PSUM bank width (f32)
NJ = N // FREE  # 4 column chunks of S per row block
DC = D // FREE  # 2 output column chunks
EPS = 1e-7
INV_SQRT_D = 1.0 / float(np.sqrt(D))

F32 = mybir.dt.float32
BF16 = mybir.dt.bfloat16
FP8 = mybir.dt.float8e4


def build_nc(finalize=True):
    # Bacc (not raw Bass): its compile() pass legalizes multi-wait
    # instructions into event semaphores, which walrus requires.
    nc = bacc.Bacc()
    # x arrives pre-cast to bf16 from the host (input marshaling; the kernel
    # computes in bf16 anyway and DMA cannot cast).
    x_ext = nc.declare_dram_parameter("x", [N, D], BF16, isOutput=False)
    m_ext = nc.declare_dram_parameter("maskf", [N], F32, isOutput=False)
    out_ext = nc.declare_dram_parameter("out", [N, D], F32, isOutput=True)

    with TileContext(nc) as tc:
        with (
            tc.tile_pool(name="persist", bufs=1) as persist,
            tc.tile_pool(name="outp", bufs=4) as outp,
            tc.tile_pool(name="psum", bufs=8, space="PSUM") as psum,
        ):
            # ---------------- constants ----------------
            ident = persist.tile([P, P], BF16, name="ident")
            make_identity(nc, ident)

            # mrow[p, t] = mask[t*P + p]
            mrow = persist.tile([P, NT], F32, name="mrow")
            nc.sync.dma_start(out=mrow, in_=m_ext[:].rearrange("(t p) -> p t", p=P))

            # additive row-mask bias for exp: 0 where mask=1, -30000 where 0
            mbias = persist.tile([P, NT], F32, name="mbias")
            nc.vector.tensor_scalar(
                out=mbias, in0=mrow, scalar1=-1.0, scalar2=30000.0,
                op0=mybir.AluOpType.add, op1=mybir.AluOpType.mult,
            )

            # column mask broadcast across partitions, cast to bf16
            # column-mask broadcast across partitions via PE outer product
            # ones[128,1] @ m[1,N] (multi-partition broadcast DMAs trip a
            # walrus "Too many sync wait commands" limit, so avoid them).
            m1 = persist.tile([1, N], F32, name="m1")
            nc.sync.dma_start(out=m1, in_=m_ext[None, :])
            m1b = persist.tile([1, N], BF16, name="m1b")
            nc.vector.tensor_copy(out=m1b, in_=m1)
            ones1 = persist.tile([1, P], BF16, name="ones1")
            nc.vector.memset(ones1, 1.0)
            mbc = persist.tile([P, N], BF16, name="mbc")
            for c in range(NJ):
                pm = psum.tile([P, FREE], F32, name="ps_m", tag="ps")
                nc.tensor.matmul(
                    pm, lhsT=ones1, rhs=m1b[:, c * FREE:(c + 1) * FREE],
                    start=True, stop=True,
                )
                nc.vector.tensor_copy(out=mbc[:, c * FREE:(c + 1) * FREE], in_=pm)

            # ---------------- persistent tensors ----------------
            xbf = persist.tile([P, NT, D], BF16, name="xbf")   # X rows, bf16
            # X^T [d, n] in fp8e4m3: scores matmul runs in DoubleRow fp8
            # (2x PE rate).  |x| <~ 5.5 << 240 so no clipping needed, and the
            # softmax-like normalization cancels the quantization of the
            # dominant diagonal term, so output error stays at bf16 level.
            xt = persist.tile([P, KT, N], FP8, name="xt")
            et = persist.tile([P, NT, N], BF16, name="et")     # masked exp(S)
            s_parts = persist.tile([P, NT, NJ], F32, name="s_parts")
            rowscale = persist.tile([P, NT], F32, name="rowscale")

            # ---- phase A: load X (bf16), transpose into xt (fp8) ----
            # 4 PE transposes share one PSUM tile; a single 3D-AP DVE copy
            # casts all 4 to fp8 (amortizes DVE per-op overhead 4x).
            for t in range(NT):
                nc.sync.dma_start(out=xbf[:, t, :], in_=x_ext[t * P:(t + 1) * P, :])
                for k2 in range(0, KT, 4):
                    pt = psum.tile([P, 4, P], BF16, name="ps_t", tag="ps")
                    for q in range(4):
                        nc.tensor.transpose(
                            pt[:, q, :], xbf[:, t, (k2 + q) * P:(k2 + q + 1) * P],
                            ident,
                        )
                    nc.vector.tensor_copy(
                        out=xt[:, k2:k2 + 4, t * P:(t + 1) * P], in_=pt
                    )

            # ---- phase B: S blocks -> exp -> col-mask + rowsum ----
            for t in range(NT):
                ps_chunks = [
                    psum.tile([P, FREE], F32, name=f"ps_s{c}", tag="ps")
                    for c in range(NJ)
                ]
                # chunk-major: each 512-wide chunk finishes its k-accumulation
                # before the next starts, so its exp/mask overlap the rest of
                # the row (PE's reorder window hides the extra LDWEIGHTS).
                for c in range(NJ):
                    for k in range(0, KT, 2):
                        nc.tensor.matmul(
                            ps_chunks[c],
                            lhsT=xt[:, k:k + 2, t * P:(t + 1) * P],
                            rhs=xt[:, k:k + 2, c * FREE:(c + 1) * FREE],
                            start=(k == 0), stop=(k == KT - 2),
                            perf_mode=mybir.MatmulPerfMode.DoubleRow,
                        )
                for c in range(NJ):
                    esl = et[:, t, c * FREE:(c + 1) * FREE]
                    nc.scalar.activation(
                        out=esl, in_=ps_chunks[c],
                        func=mybir.ActivationFunctionType.Exp,
                        bias=mbias[:, t:t + 1], scale=INV_SQRT_D,
                    )
                    # (tensor_tensor_reduce is rejected by this runtime;
                    # InstTensorScalarPtr with is_scalar_tensor_tensor works)
                    nc.vector.scalar_tensor_tensor(
                        out=esl, in0=esl, scalar=1.0,
                        in1=mbc[:, c * FREE:(c + 1) * FREE],
                        op0=mybir.AluOpType.bypass,
                        op1=mybir.AluOpType.mult,
                        accum_out=s_parts[:, t, c:c + 1],
                    )
                nc.vector.reduce_sum(
                    out=rowscale[:, t:t + 1], in_=s_parts[:, t, :],
                    axis=mybir.AxisListType.X,
                )
                nc.vector.tensor_scalar_add(
                    out=rowscale[:, t:t + 1], in0=rowscale[:, t:t + 1], scalar1=EPS
                )
                nc.vector.reciprocal(
                    out=rowscale[:, t:t + 1], in_=rowscale[:, t:t + 1]
                )

            # ---- phase C: U = E' @ X, row-scale, store ----
            for b in range(NT):
                for c in range(DC):
                    pu = psum.tile([P, FREE], F32, name="ps_u", tag="ps")
                    for a in range(NT):
                        nc.tensor.matmul(
                            pu, lhsT=et[:, a, b * P:(b + 1) * P],
                            rhs=xbf[:, a, c * FREE:(c + 1) * FREE],
                            start=(a == 0), stop=(a == NT - 1),
                        )
                    ot = outp.tile([P, FREE], F32, name="ot", tag="ot")
                    nc.vector.tensor_scalar_mul(
                        out=ot, in0=pu, scalar1=rowscale[:, b:b + 1]
                    )
                    nc.gpsimd.dma_start(
                        out=out_ext[b * P:(b + 1) * P, c * FREE:(c + 1) * FREE],
                        in_=ot,
                    )
    if finalize:
        nc.finalize()
    return nc


_RUNNER = None


def _make_runner(nc=None):
    """Compile the SPMD NEFF once; return f(x2d, m1d, zeros) -> out2d.

    Mirrors concourse.bass2jax.run_bass_via_pjrt's multi-core path (shard_map
    over 8 cores, per-core shard = BIR-declared shape), but keeps the jitted
    callable so repeat calls don't retrace/recompile, and skips output-buffer
    donation (this kernel writes every output element).
    """
    import jax
    from jax.sharding import Mesh, PartitionSpec
    from jax.experimental.shard_map import shard_map
    import concourse.mybir as mybir
    from concourse import bass2jax

    bass2jax.install_neuronx_cc_hook()
    if nc is None:
        nc = build_nc()
    assert nc.dbg_addr is None
    partition_name = nc.partition_id_tensor.name if nc.partition_id_tensor else None

    in_names, out_names, out_avals = [], [], []
    for alloc in nc.m.functions[0].allocations:
        if not isinstance(alloc, mybir.MemoryLocationSet):
            continue
        name = alloc.memorylocations[0].name
        if alloc.kind == "ExternalInput":
            if name != partition_name:
                in_names.append(name)
        elif alloc.kind == "ExternalOutput":
            out_names.append(name)
            out_avals.append(
                jax.core.ShapedArray(tuple(alloc.tensor_shape), mybir.dt.np(alloc.dtype))
            )
    n_params = len(in_names)
    all_names = in_names + out_names
    if partition_name is not None:
        all_names = all_names + [partition_name]

    def _body(*args):
        operands = list(args)
        if partition_name is not None:
            operands.append(bass2jax.partition_id_tensor())
        outs = bass2jax._bass_exec_p.bind(
            *operands,
            out_avals=tuple(out_avals),
            in_names=tuple(all_names),
            out_names=tuple(out_names),
            lowering_input_output_aliases=(),
            sim_require_finite=True,
            sim_require_nnan=True,
            nc=nc,
        )
        return tuple(outs)

    devices = jax.devices()[:B]
    mesh = Mesh(np.asarray(devices), ("core",))
    n_args = n_params + len(out_names)
    sharded = jax.jit(
        shard_map(
            _body,
            mesh=mesh,
            in_specs=(PartitionSpec("core"),) * n_args,
            out_specs=(PartitionSpec("core"),) * len(out_names),
            check_rep=False,
        ),
        keep_unused=True,
    )
    zeros = [np.zeros((B * a.shape[0], *a.shape[1:]), a.dtype) for a in out_avals]
    return sharded, zeros, [tuple(a.shape) for a in out_avals], in_names, mesh


def _get_runner():
    global _RUNNER
    if _RUNNER is None:
        _RUNNER = _make_runner()
    return _RUNNER


def _make_runner_for(nc):
    """Timing helper for test.py: runner for an alternate prebuilt graph."""
    sharded, _zeros, _shapes, _names, _mesh = _make_runner(nc)
    return sharded


def make_chain_runner(chain_k):
    """Timing helper: one jitted dispatch that executes the NEFF chain_k times
    sequentially (scalar data dependency between iterations prevents
    reordering/DCE), so per-exec device time = slope over chain_k."""
    import jax
    import jax.numpy as jnp
    from jax.sharding import Mesh, PartitionSpec
    from jax.experimental.shard_map import shard_map
    import concourse.mybir as mybir
    from concourse import bass2jax

    bass2jax.install_neuronx_cc_hook()
    nc = build_nc()
    partition_name = nc.partition_id_tensor.name if nc.partition_id_tensor else None
    in_names, out_names, out_avals = [], [], []
    for alloc in nc.m.functions[0].allocations:
        if not isinstance(alloc, mybir.MemoryLocationSet):
            continue
        name = alloc.memorylocations[0].name
        if alloc.kind == "ExternalInput":
            if name != partition_name:
                in_names.append(name)
        elif alloc.kind == "ExternalOutput":
            out_names.append(name)
            out_avals.append(
                jax.core.ShapedArray(tuple(alloc.tensor_shape), mybir.dt.np(alloc.dtype))
            )
    all_names = in_names + out_names
    if partition_name is not None:
        all_names = all_names + [partition_name]

    def _bind(x, m, z):
        operands = [x, m, z]
        if partition_name is not None:
            operands.append(bass2jax.partition_id_tensor())
        return bass2jax._bass_exec_p.bind(
            *operands,
            out_avals=tuple(out_avals),
            in_names=tuple(all_names),
            out_names=tuple(out_names),
            lowering_input_output_aliases=(),
            sim_require_finite=True,
            sim_require_nnan=True,
            nc=nc,
        )

    def _body(x, m, z):
        out = None
        for _ in range(chain_k):
            (out,) = _bind(x, m, z)
            # runtime-zero scalar dep (not foldable at compile time)
            dep = jnp.where(jnp.isnan(out[0, 0]), 1.0, 0.0).astype(x.dtype)
            x = x + dep
        return (out,)

    devices = jax.devices()[:B]
    mesh = Mesh(np.asarray(devices), ("core",))
    sharded = jax.jit(
        shard_map(
            _body, mesh=mesh,
            in_specs=(PartitionSpec("core"),) * 3,
            out_specs=(PartitionSpec("core"),),
            check_rep=False,
        ),
        keep_unused=True,
    )
    zeros = [np.zeros((B * a.shape[0], *a.shape[1:]), a.dtype) for a in out_avals]
    return sharded, zeros, mesh, in_names


def _prep(x, mask):
    import ml_dtypes

    xb = np.ascontiguousarray(
        np.asarray(x, dtype=np.float32).astype(ml_dtypes.bfloat16)
    )
    maskf = np.ascontiguousarray(np.asarray(mask).astype(np.float32))
    assert xb.shape == (B, N, D) and maskf.shape == (B, N)
    # per-core shard of axis 0: concat over cores = just the 2D views
    return {"x": xb.reshape(B * N, D), "maskf": maskf.reshape(B * N)}


def kernel(x, mask):
    sharded, zeros, out_shapes, in_names, _mesh = _get_runner()
    ins = _prep(x, mask)
    out_arrs = sharded(*[ins[n] for n in in_names], *zeros)
    out = np.asarray(out_arrs[0]).reshape(B, *out_shapes[0])
    return out
